# revision 13
# baseline (speedup 1.0000x reference)
"""DGP loss kernel for Trainium2, 8 NeuronCores, pure data parallel.

Math (algebraically identical to the reference):
  - The reference extracts overlapping 5x5 patches (stride 1) of the 4x-downsampled
    depth grid and takes a masked lower-median over each 4x4 depth sub-block.
    Sub-block (u,v) of patch (a,b) is exactly the aligned 4x4 depth block at
    feature-grid cell (a+u, b+v), so we compute the median once per cell:
    M[i,j], i<128, j<256.
  - seg branch: with sf = F.normalize(seg_feat, dim=C), the patch term is
    ||sf_c - sf_n||^2 = 2 - 2*dot(sf_c, sf_n), so
    loss_term(center,offset) = exp(-|M_c - M_n|) * exp(2*dot - 2)
    summed over centers i in [2,126), j in [2,254) and the 24 offsets
    (du,dv) in [-2,2]^2 minus (0,0); valid_amount = sum (M_c>0)&(M_n>0).
  - per_img = sum(terms)/max(valid_amount,1); loss = mean over images.

Sharding: 8 cores = 2 images x 4 column bands (63 centers each, +2 halo).

Layouts (engine APs must start at partition 0/32/64/96, so row shifts must
live in the free dimension):
  - depth/median work: [i=128 partitions, free] (no shifted operands needed)
  - correlation: sf_strip [(s:4 x c:32)=128 partitions, (ii:35, j:67) free],
    strip s holds feature rows 31s..31s+35; both du and dv shifts are free-dim
    offsets. Channel reduction via PE matmul with a block-diagonal ones lhsT
    -> PSUM [4, 31*63]; a PSUM->SBUF DMA regroups (s, ii) -> center row.
  - shifted copies of the small per-cell tensors M, rinv (5 row shifts each)
    are made with SBUF->SBUF DMAs (DMA has no partition-start restriction).
"""

from contextlib import ExitStack

import numpy as np

import concourse.bass as bass
import concourse.mybir as mybir
import concourse.tile as tile
from concourse import bass_utils
from concourse.alu_op_type import AluOpType

F32 = mybir.dt.float32
AF = mybir.ActivationFunctionType

EPS_FN = 1e-8
BIG = 3.0e38
MIN_D, MAX_D = 1.0, 100.0
NSCALE = 1.0 / (1.0 / MIN_D - 1.0 / MAX_D)          # 1/0.99
NBIAS = -(1.0 / MAX_D) * NSCALE                      # -0.01/0.99

# Per-core shard geometry (2 images x 4 bands of 63 centers).
JBAND = 63          # centers per band
JCOLS = JBAND + 4   # feature cols loaded (halo 2 each side)
DCOLS = 4 * JCOLS   # depth cols loaded
NROW = 35           # feature rows per strip (31 centers + 4 halo)
NCEN = 31           # center rows per strip


def _oddeven_merge_sort_layers(n):
    """Batcher odd-even mergesort compare-exchange pairs, grouped by layer."""
    layers = []
    p = 1
    while p < n:
        k = p
        while k >= 1:
            layer = []
            for j in range(k % p, n - k, 2 * k):
                for i in range(0, min(k, n - j - k)):
                    if (i + j) // (p * 2) == (i + j + k) // (p * 2):
                        layer.append((i + j, i + j + k))
            layers.append(layer)
            k //= 2
        p *= 2
    return layers


def _group_runs(pairs):
    """Group CE pairs (a, a+d) of one layer into (a0, step, count, d) runs
    with arithmetic-progression a's so each run is one strided AP op."""
    by_d = {}
    for a, b in pairs:
        by_d.setdefault(b - a, []).append(a)
    runs = []
    for d, alist in sorted(by_d.items()):
        alist = sorted(alist)
        i = 0
        while i < len(alist):
            j = i + 1
            step = None
            while j < len(alist):
                s = alist[j] - alist[j - 1]
                if step is None:
                    step = s
                elif s != step:
                    break
                j += 1
            cnt = j - i
            runs.append((alist[i], step if cnt > 1 else 1, cnt, d))
            i = j
    return runs


def _planes(t, start, step, count):
    """AP over plane dim of a [128, NP, W] tile: planes start, start+step, ..."""
    if count == 1:
        return t[:, start : start + 1, :]
    return t[:, start : start + (count - 1) * step + 1 : step, :]


def _row_to_strip(r):
    """Canonical (strip, row-in-strip) for a global feature row r."""
    s = min(r // NCEN, 3)
    return s, r - NCEN * s


def _split_excess_waits(nc, max_waits=1):
    """This container's walrus build rejects instructions carrying more than
    one sem-wait ("Too many sync wait commands"); Tile's scheduler happily
    attaches several. Move the excess onto standalone EventSemaphore waits
    immediately before the instruction on the same engine queue."""
    for f in nc.m.functions:
        for blk in f.blocks:
            new_insts = []
            for inst in blk.instructions:
                si = inst.sync_info
                if si is not None and si.on_wait and len(si.on_wait) > max_waits:
                    waits = list(si.on_wait)
                    excess, keep = waits[:-max_waits], waits[-max_waits:]
                    idx = 0
                    while excess:
                        chunk, excess = excess[:max_waits], excess[max_waits:]
                        new_insts.append(
                            mybir.InstEventSemaphore(
                                name=f"{inst.name}-wsplit{idx}",
                                engine=inst.engine,
                                ins=[],
                                outs=[],
                                sync_info=mybir.SyncInfo(on_wait=chunk, on_update=[]),
                            )
                        )
                        idx += 1
                    si.on_wait = keep
                new_insts.append(inst)
            blk.instructions[:] = new_insts


def _build_core_program(split_waits=True):
    nc = bass.Bass("TRN2", target_bir_lowering=False, debug=False)
    dep = nc.dram_tensor("dep", [512, DCOLS], F32, kind="ExternalInput")
    sf = nc.dram_tensor("sf", [32, 128, JCOLS], F32, kind="ExternalInput")
    out = nc.dram_tensor("out", [124, 4], F32, kind="ExternalOutput")

    with tile.TileContext(nc) as tc, ExitStack() as ctx:
        persist = ctx.enter_context(tc.tile_pool(name="persist", bufs=1))
        work = ctx.enter_context(tc.tile_pool(name="work", bufs=1))
        prods = ctx.enter_context(tc.tile_pool(name="prods", bufs=3))

        v = nc.vector
        act = nc.scalar

        # ---------------- depth branch ([i=128, ...] layout) ----------------
        dep_raw = work.tile([128, 4, DCOLS], F32)
        nc.sync.dma_start(
            out=dep_raw, in_=dep.ap().rearrange("(i r) w -> i r w", r=4)
        )
        dcl = work.tile([128, 4, DCOLS], F32)
        v.tensor_scalar(dcl, dep_raw, MIN_D, MAX_D, op0=AluOpType.max, op1=AluOpType.min)
        vld = work.tile([128, 4, DCOLS], F32)
        v.tensor_tensor(vld, dcl, dep_raw, op=AluOpType.is_equal)
        rec = work.tile([128, 4, DCOLS], F32)
        v.reciprocal(rec, dcl)
        aff = work.tile([128, 4, DCOLS], F32)
        act.activation(aff, rec, AF.Copy, bias=NBIAS, scale=NSCALE)
        dep_n = work.tile([128, 4, DCOLS], F32)
        v.tensor_tensor(dep_n, aff, vld, op=AluOpType.mult)

        # invalid -> +BIG additive mask
        nb = work.tile([128, 4, DCOLS], F32)
        v.tensor_scalar(nb, dep_n, EPS_FN, BIG, op0=AluOpType.is_le, op1=AluOpType.mult)

        # valid count k per 4x4 block
        msk = work.tile([128, 4, DCOLS], F32)
        v.tensor_scalar(msk, dep_n, EPS_FN, None, op0=AluOpType.is_gt)
        k_t = persist.tile([128, JCOLS], F32)
        v.reduce_sum(
            out=k_t,
            in_=msk.rearrange("p r (j s) -> p j r s", s=4),
            axis=mybir.AxisListType.XY,
        )

        # sort planes S[m=(r*4+s), j] ascending with invalid -> ~BIG
        S = persist.tile([128, 16, JCOLS], F32)
        v.tensor_tensor(
            out=S.rearrange("p (r s) j -> p r j s", s=4),
            in0=dep_n.rearrange("p r (j s) -> p r j s", s=4),
            in1=nb.rearrange("p r (j s) -> p r j s", s=4),
            op=AluOpType.add,
        )
        for layer in _oddeven_merge_sort_layers(16):
            for a0, astep, cnt, d in _group_runs(layer):
                lo = _planes(S, a0, astep, cnt)
                hi = _planes(S, a0 + d, astep, cnt)
                tmp = prods.tile([128, cnt, JCOLS], F32, tag="cetmp")
                v.tensor_tensor(tmp[:, :cnt, :], lo, hi, op=AluOpType.max)
                v.tensor_tensor(lo, lo, hi, op=AluOpType.min)
                v.tensor_copy(hi, tmp[:, :cnt, :])

        # lower-median select: u_m = [k>=2m+1] - [k>=2m+3], m=0..7
        G = work.tile([128, 9, JCOLS], F32)
        for m in range(9):
            v.tensor_scalar(G[:, m, :], k_t, float(2 * m + 1), None, op0=AluOpType.is_ge)
        u = work.tile([128, 8, JCOLS], F32)
        v.tensor_tensor(u, G[:, 0:8, :], G[:, 1:9, :], op=AluOpType.subtract)
        sel = work.tile([128, 8, JCOLS], F32)
        v.tensor_tensor(sel, S[:, 0:8, :], u, op=AluOpType.mult)
        M = persist.tile([128, JCOLS], F32)
        v.reduce_sum(
            out=M, in_=sel.rearrange("p m j -> p j m"), axis=mybir.AxisListType.X
        )

        # ---------------- seg branch (strip layout) ----------------
        sf_strip = persist.tile([128, NROW, JCOLS], F32)
        for s in range(4):
            nc.sync.dma_start(
                out=sf_strip[32 * s : 32 * (s + 1), :, :],
                in_=sf.ap()[:, NCEN * s : NCEN * s + NROW, :],
            )

        # block-diagonal ones for the strip-wise channel reduction
        ones4 = persist.tile([128, 4], F32)
        v.memset(ones4, 0.0)
        for s in range(4):
            v.memset(ones4[32 * s : 32 * (s + 1), s : s + 1], 1.0)

        # squared-norm and 1/||f|| per cell, in strip layout
        f2 = work.tile([128, NROW, JCOLS], F32)
        v.tensor_tensor(f2, sf_strip, sf_strip, op=AluOpType.mult)
        NF = NROW * JCOLS  # 2345
        nrm = work.tile([4, NF], F32)
        with tc.tile_pool(name="psnrm", bufs=1, space="PSUM") as psnrm:
            nrm2_ps = psnrm.tile([4, NF], F32, tag="nrm2")
            for c0 in range(0, NF, 512):
                cw = min(512, NF - c0)
                nc.tensor.matmul(
                    nrm2_ps[:, c0 : c0 + cw],
                    ones4,
                    f2.rearrange("p a b -> p (a b)")[:, c0 : c0 + cw],
                )
            act.activation(nrm, nrm2_ps, AF.Sqrt)
        rinv_strip = persist.tile([4, NROW, JCOLS], F32)
        v.reciprocal(rinv_strip.rearrange("p a b -> p (a b)"), nrm)

        # shifted copies of rinv and M: xx_sh[d][p, j] = xx[row p+d, col j]
        # (rinv rows come from the canonical strip; M rows from the i-layout M)
        rinv_sh = []
        M_sh = []
        for d in range(5):
            rt = persist.tile([124, JCOLS], F32, tag=f"rinv_sh{d}")
            mt = persist.tile([124, JCOLS], F32, tag=f"M_sh{d}")
            # group dst rows p by the canonical strip of row r = p + d
            p0 = 0
            while p0 < 124:
                s, ii = _row_to_strip(p0 + d)
                pmax = min(124, NCEN * (s + 1) - d) if s < 3 else 124
                cnt = pmax - p0
                nc.sync.dma_start(
                    out=rt[p0:pmax, :],
                    in_=rinv_strip[s : s + 1, ii : ii + cnt, :],
                )
                p0 = pmax
            nc.sync.dma_start(out=mt, in_=M[d : d + 124, :])
            rinv_sh.append(rt)
            M_sh.append(mt)

        # ---------------- 25-offset correlation ----------------
        psum = ctx.enter_context(tc.tile_pool(name="psum", bufs=2, space="PSUM"))
        dots = persist.tile([124, 25, JBAND], F32)
        dda = persist.tile([124, 25, JBAND], F32)
        vm = persist.tile([124, 25, JBAND], F32)

        cen = sf_strip[:, 2 : 2 + NCEN, 2 : 2 + JBAND]
        NP = NCEN * JBAND  # 1953
        for o in range(25):
            du, dv = o // 5, o % 5
            par = sf_strip[:, du : du + NCEN, dv : dv + JBAND]
            prod = prods.tile([128, NCEN, JBAND], F32, tag="prod")
            v.tensor_tensor(prod, cen, par, op=AluOpType.mult)
            dps = psum.tile([4, NP], F32, tag="dps")
            for c0 in range(0, NP, 512):
                cw = min(512, NP - c0)
                nc.tensor.matmul(
                    dps[:, c0 : c0 + cw],
                    ones4,
                    prod.rearrange("p a b -> p (a b)")[:, c0 : c0 + cw],
                )
            # regroup PSUM [s, (ii, j)] -> dots[p = s*31+ii, o, j]
            # (DMA cannot read PSUM: bounce via an ACT copy to SBUF first)
            dsb = prods.tile([4, NP], F32, tag="dsb")
            act.activation(dsb, dps, AF.Copy)
            nc.sync.dma_start(
                out=dots[:, o, :],
                in_=dsb.rearrange("s (a b) -> s a b", a=NCEN),
            )

            rc = rinv_sh[2][:, 2 : 2 + JBAND]
            v.tensor_tensor(dots[:, o, :], dots[:, o, :], rc, op=AluOpType.mult)
            v.tensor_tensor(
                dots[:, o, :],
                dots[:, o, :],
                rinv_sh[du][:, dv : dv + JBAND],
                op=AluOpType.mult,
            )
            Mc = M_sh[2][:, 2 : 2 + JBAND]
            Mn = M_sh[du][:, dv : dv + JBAND]
            v.tensor_tensor(dda[:, o, :], Mc, Mn, op=AluOpType.subtract)
            v.tensor_tensor(vm[:, o, :], Mc, Mn, op=AluOpType.mult)

        act.activation(dda, dda, AF.Abs)
        v.tensor_scalar(vm, vm, 0.0, None, op0=AluOpType.is_gt)

        xt = work.tile([124, 25, JBAND], F32)
        v.scalar_tensor_tensor(
            xt, dots, 2.0, dda, op0=AluOpType.mult, op1=AluOpType.subtract
        )
        bias_m2 = persist.tile([124, 1], F32)
        v.memset(bias_m2, -2.0)
        terms = work.tile([124, 25, JBAND], F32)
        act.activation(terms, xt, AF.Exp, bias=bias_m2, scale=1.0)

        numden = work.tile([124, 4], F32)
        v.reduce_sum(out=numden[:, 0:1], in_=terms[:, 0:12, :], axis=mybir.AxisListType.XY)
        v.reduce_sum(out=numden[:, 1:2], in_=terms[:, 13:25, :], axis=mybir.AxisListType.XY)
        v.reduce_sum(out=numden[:, 2:3], in_=vm[:, 0:12, :], axis=mybir.AxisListType.XY)
        v.reduce_sum(out=numden[:, 3:4], in_=vm[:, 13:25, :], axis=mybir.AxisListType.XY)
        nc.sync.dma_start(out=out.ap(), in_=numden)

    if split_waits:
        _split_excess_waits(nc)
    return nc


_NC_CACHE = []


def kernel(seg_feat: np.ndarray, dep_true: np.ndarray) -> np.ndarray:
    seg_feat = np.ascontiguousarray(seg_feat, dtype=np.float32)
    dep_true = np.ascontiguousarray(dep_true, dtype=np.float32)

    if not _NC_CACHE:
        _NC_CACHE.append(_build_core_program())
    nc = _NC_CACHE[0]

    in_maps = []
    for core in range(8):
        img, band = core // 4, core % 4
        j0 = JBAND * band
        in_maps.append(
            {
                "dep": np.ascontiguousarray(dep_true[img, :, 4 * j0 : 4 * j0 + DCOLS]),
                "sf": np.ascontiguousarray(seg_feat[img, :, :, j0 : j0 + JCOLS]),
            }
        )

    res = bass_utils.run_bass_kernel_spmd(nc, in_maps, core_ids=list(range(8)))
    parts = [r["out"].astype(np.float64) for r in res.results]

    loss = 0.0
    for img in range(2):
        num = sum(parts[img * 4 + b][:, 0:2].sum() for b in range(4))
        den = sum(parts[img * 4 + b][:, 2:4].sum() for b in range(4))
        loss += num / max(den, 1.0)
    return np.float32(loss / 2.0)
